# revision 22
# baseline (speedup 1.0000x reference)
import sys

sys.path.insert(0, "/opt/trn_rl_repo")

import numpy as np

B = 16
N_CORES = 8
B_LOC = B // N_CORES  # 2 samples per core
H1, H2 = 128, 64
DIM_CH = [3, 64, 128, 256, 512]
CS = [64, 128, 256, 512, 512]
FS = [(1 + c) * 9 for c in DIM_CH]  # [36, 585, 1161, 2313, 4617]
DINS = [FS[0]] + [FS[i] + CS[i - 1] * H2 for i in range(1, 5)]
NCH = [(f + 127) // 128 for f in FS]  # [1, 5, 10, 19, 37]
BASE = [2 * sum(NCH[:i]) for i in range(5)]
TOT_COLS = 2 * sum(NCH)  # 144

_CACHE = {}


def _build():
    from contextlib import ExitStack

    import concourse.bacc as bacc
    import concourse.tile as tile
    from concourse import mybir

    f32 = mybir.dt.float32
    f32r = mybir.dt.float32r
    AF = mybir.ActivationFunctionType
    ALU = mybir.AluOpType
    AX = mybir.AxisListType

    nc = bacc.Bacc(None, target_bir_lowering=False)

    p_dram = [
        nc.dram_tensor(f"p{i}", [B_LOC, FS[i], CS[i]], f32, kind="ExternalInput")
        for i in range(5)
    ]
    w1_dram = [
        nc.dram_tensor(f"W1_{i}", [DINS[i], H1], f32, kind="ExternalInput")
        for i in range(5)
    ]
    b1_dram = [
        nc.dram_tensor(f"b1_{i}", [H1], f32, kind="ExternalInput") for i in range(5)
    ]
    w2_dram = [
        nc.dram_tensor(f"W2_{i}", [H1, H2], f32, kind="ExternalInput")
        for i in range(5)
    ]
    b2_dram = [
        nc.dram_tensor(f"b2_{i}", [H2], f32, kind="ExternalInput") for i in range(5)
    ]
    wr_dram = nc.dram_tensor("Wr", [5 * H2 + 1, 1], f32, kind="ExternalInput")
    br_dram = nc.dram_tensor("br", [1], f32, kind="ExternalInput")
    out_dram = nc.dram_tensor("out", [B_LOC, 1], f32, kind="ExternalOutput")

    with tile.TileContext(nc) as tc, ExitStack() as ctx:
        constp = ctx.enter_context(tc.tile_pool(name="const", bufs=1))
        xpool = ctx.enter_context(tc.tile_pool(name="x", bufs=8))
        wfpool = ctx.enter_context(tc.tile_pool(name="wf", bufs=4))
        wppool = ctx.enter_context(tc.tile_pool(name="wp", bufs=6))
        hpool = ctx.enter_context(tc.tile_pool(name="h", bufs=3))
        h2pool = ctx.enter_context(tc.tile_pool(name="h2", bufs=3))
        fpool = ctx.enter_context(tc.tile_pool(name="f", bufs=2))
        smallp = ctx.enter_context(tc.tile_pool(name="small", bufs=2))
        psump = ctx.enter_context(tc.tile_pool(name="ps", bufs=1, space="PSUM"))

        ones_col = constp.tile([128, 1], f32, tag="ones_col", name="ones_col")
        nc.vector.memset(ones_col[:], 1.0)
        ones_row = constp.tile([1, 128], f32, tag="ones_row", name="ones_row")
        nc.vector.memset(ones_row[:], 1.0)
        stats = constp.tile([128, TOT_COLS], f32, tag="stats", name="stats")
        nc.vector.memset(stats[:], 0.0)
        reps = constp.tile([64, 10], f32, tag="reps", name="reps")
        L_all = constp.tile([1, 10], f32, tag="L_all", name="L_all")
        sqscr = constp.tile([128, 512], f32, tag="sqscr", name="sqscr")

        b1_sb, b2d_sb, w2d_sb = [], [], []
        for i in range(5):
            b1t = constp.tile([H1, 1], f32, tag=f"b1_{i}", name=f"b1s_{i}")
            nc.sync.dma_start(b1t[:], b1_dram[i][:])
            b1_sb.append(b1t)
            b2t = constp.tile([128, 1], f32, tag=f"b2_{i}", name=f"b2s_{i}")
            nc.sync.dma_start(b2t[0:64, :], b2_dram[i][:])
            nc.sync.dma_start(b2t[64:128, :], b2_dram[i][:])
            b2d_sb.append(b2t)
            w2t = constp.tile([H1, 128], f32, tag=f"w2_{i}", name=f"w2s_{i}")
            nc.sync.dma_start(w2t[:, 0:64].bitcast(f32r), w2_dram[i][:].bitcast(f32r))
            nc.sync.dma_start(w2t[:, 64:128].bitcast(f32r), w2_dram[i][:].bitcast(f32r))
            w2d_sb.append(w2t)

        f_prev = None
        for i in range(5):
            C, F, nch = CS[i], FS[i], NCH[i]
            packed = 2 * C <= 512

            # ---- prev-features matmuls (i > 0): prevT = W1p.T @ f_prev ----
            prevps = None
            if i > 0:
                M_in = CS[i - 1] * H2 // 128
                prevps = psump.tile(
                    [128, 512], f32, tag="prev", bufs=2, name=f"prevps_{i}"
                )
                fv = f_prev.rearrange("p (b m) -> p m b", b=2)
                for t in range(M_in // 4):
                    wp4 = wppool.tile([128, 512], f32, tag="wp4", name=f"wp4_{i}_{t}")
                    nc.sync.dma_start(
                        wp4[:].bitcast(f32r).rearrange("p (j h) -> p j h", j=4),
                        w1_dram[i][F + t * 512 : F + (t + 1) * 512, :]
                        .bitcast(f32r)
                        .rearrange("(j p) h -> p j h", j=4),
                    )
                    for j in range(4):
                        m = t * 4 + j
                        nc.tensor.matmul(
                            prevps[0:128, 0:2],
                            wp4[:, j * 128 : (j + 1) * 128].bitcast(f32r),
                            fv[:, m : m + 1, :].bitcast(f32r),
                            start=(m == 0),
                            stop=(m == M_in - 1),
                        )

            # ---- z matmuls (accumulate over F chunks) + sumsq stats ----
            if packed:
                zps = psump.tile([128, 512], f32, tag="z", bufs=3, name=f"zps_{i}")
                for k in range(nch):
                    kl = min(128, F - k * 128)
                    xt = xpool.tile([128, 2 * C], f32, tag="x", name=f"x_{i}_{k}")
                    for b in range(2):
                        nc.sync.dma_start(
                            xt[0:kl, b * C : (b + 1) * C].bitcast(f32r),
                            p_dram[i][b, k * 128 : k * 128 + kl, :].bitcast(f32r),
                        )
                    wf = wfpool.tile([128, 128], f32, tag="wf", name=f"wf_{i}_{k}")
                    nc.sync.dma_start(
                        wf[0:kl, :].bitcast(f32r),
                        w1_dram[i][k * 128 : k * 128 + kl, :].bitcast(f32r),
                    )
                    nc.tensor.matmul(
                        zps[0:128, 0 : 2 * C],
                        wf[0:kl, :].bitcast(f32r),
                        xt[0:kl, :].bitcast(f32r),
                        start=(k == 0),
                        stop=(k == nch - 1),
                    )
                    for b in range(2):
                        col = BASE[i] + b * nch + k
                        nc.scalar.activation(
                            sqscr[0:kl, 0:C],
                            xt[0:kl, b * C : (b + 1) * C],
                            AF.Square,
                            accum_out=stats[0:kl, col : col + 1],
                        )
                z_list = [(zps, 0), (zps, C)]
            else:
                zpsA = psump.tile([128, 512], f32, tag="z", bufs=3, name=f"zpsA_{i}")
                zpsB = psump.tile([128, 512], f32, tag="z", bufs=3, name=f"zpsB_{i}")
                for k in range(nch):
                    kl = min(128, F - k * 128)
                    wf = wfpool.tile([128, 128], f32, tag="wf", name=f"wf_{i}_{k}")
                    nc.sync.dma_start(
                        wf[0:kl, :].bitcast(f32r),
                        w1_dram[i][k * 128 : k * 128 + kl, :].bitcast(f32r),
                    )
                    for b, zp in ((0, zpsA), (1, zpsB)):
                        xt = xpool.tile([128, 512], f32, tag="x", name=f"x_{i}_{k}_{b}")
                        nc.sync.dma_start(
                            xt[0:kl, 0:C].bitcast(f32r),
                            p_dram[i][b, k * 128 : k * 128 + kl, :].bitcast(f32r),
                        )
                        nc.tensor.matmul(
                            zp[0:128, 0:C],
                            wf[0:kl, :].bitcast(f32r),
                            xt[0:kl, 0:C].bitcast(f32r),
                            start=(k == 0),
                            stop=(k == nch - 1),
                        )
                        col = BASE[i] + b * nch + k
                        nc.scalar.activation(
                            sqscr[0:kl, 0:C],
                            xt[0:kl, 0:C],
                            AF.Square,
                            accum_out=stats[0:kl, col : col + 1],
                        )
                z_list = [(zpsA, 0), (zpsB, 0)]

            # ---- scale factors: sumsq -> ln -> exp(-0.5 ln) broadcast ----
            st2 = smallp.tile([128, 2], f32, tag="st2", name=f"st2_{i}")
            for b in range(2):
                nc.vector.tensor_reduce(
                    st2[:, b : b + 1],
                    stats[:, BASE[i] + b * nch : BASE[i] + (b + 1) * nch],
                    axis=AX.X,
                    op=ALU.add,
                )
            sumps = psump.tile([1, 512], f32, tag="small", name=f"sumps_{i}")
            nc.tensor.matmul(
                sumps[0:1, 0:2], ones_col[:], st2[:], start=True, stop=True
            )
            nc.scalar.activation(L_all[0:1, 2 * i : 2 * i + 2], sumps[0:1, 0:2], AF.Ln)
            inv_sf = smallp.tile([1, 2], f32, tag="invsf", name=f"invsf_{i}")
            nc.scalar.activation(
                inv_sf[:], L_all[0:1, 2 * i : 2 * i + 2], AF.Exp, scale=-0.5
            )
            bcps = psump.tile([128, 512], f32, tag="small", name=f"bcps_{i}")
            nc.tensor.matmul(
                bcps[0:128, 0:2], ones_row[:], inv_sf[:], start=True, stop=True
            )
            inv_bc = smallp.tile([128, 2], f32, tag="invbc", name=f"invbc_{i}")
            nc.vector.tensor_copy(inv_bc[:], bcps[0:128, 0:2])

            # ---- bias = b1 (+ prevT) ----
            if i > 0:
                btot = smallp.tile([128, 2], f32, tag="btot", name=f"btot_{i}")
                nc.vector.tensor_scalar_add(btot[:], prevps[0:128, 0:2], b1_sb[i][:])
                bias_aps = [btot[:, 0:1], btot[:, 1:2]]
            else:
                bias_aps = [b1_sb[0][:], b1_sb[0][:]]

            # ---- h = relu(z/sf + bias) ----
            if packed:
                ht = hpool.tile([128, 2 * C], f32, tag="h", name=f"ht_{i}")
                for b in range(2):
                    nc.scalar.activation(
                        ht[:, b * C : (b + 1) * C].bitcast(f32r),
                        zps[0:128, b * C : (b + 1) * C],
                        AF.Relu,
                        bias=bias_aps[b],
                        scale=inv_bc[:, b : b + 1],
                    )
                h_list = [(ht, 0), (ht, C)]
            else:
                htA = hpool.tile([128, 512], f32, tag="h", name=f"htA_{i}")
                htB = hpool.tile([128, 512], f32, tag="h", name=f"htB_{i}")
                for b, h in ((0, htA), (1, htB)):
                    zp, off = z_list[b]
                    nc.scalar.activation(
                        h[:, 0:C].bitcast(f32r),
                        zp[0:128, off : off + C],
                        AF.Relu,
                        bias=bias_aps[b],
                        scale=inv_bc[:, b : b + 1],
                    )
                h_list = [(htA, 0), (htB, 0)]

            # ---- h2 = relu(h @ W2 + b2), with duplicated W2 for fold ----
            if packed:
                w2ps = psump.tile([128, 512], f32, tag="w2", bufs=2, name=f"w2ps_{i}")
                nc.tensor.matmul(
                    w2ps[0:128, 0 : 2 * C],
                    w2d_sb[i][:].bitcast(f32r),
                    ht[:, 0 : 2 * C].bitcast(f32r),
                    start=True,
                    stop=True,
                )
                w2_list = [(w2ps, 0), (w2ps, C)]
            else:
                w2psA = psump.tile([128, 512], f32, tag="w2", bufs=2, name=f"w2psA_{i}")
                w2psB = psump.tile([128, 512], f32, tag="w2", bufs=2, name=f"w2psB_{i}")
                for b, wp in ((0, w2psA), (1, w2psB)):
                    h, off = h_list[b]
                    nc.tensor.matmul(
                        wp[0:128, 0:C],
                        w2d_sb[i][:].bitcast(f32r),
                        h[:, off : off + C].bitcast(f32r),
                        start=True,
                        stop=True,
                    )
                w2_list = [(w2psA, 0), (w2psB, 0)]

            # ---- relu + reps + fold into next-layer prev features ----
            f_next = None
            if i < 4:
                M_out = C // 2
                f_next = fpool.tile([128, 2 * M_out], f32, tag="f", name=f"f_{i}")
            for b in range(2):
                w2t, off = w2_list[b]
                if i < 4:
                    h2d = h2pool.tile([128, 512], f32, tag="h2", name=f"h2_{i}_{b}")
                    nc.scalar.activation(
                        h2d[:, 0:C],
                        w2t[0:128, off : off + C],
                        AF.Relu,
                        bias=b2d_sb[i][:],
                    )
                    ev = h2d[0:64, 0:C].rearrange("p (m two) -> p m two", two=2)
                    od = h2d[64:128, 0:C].rearrange("p (m two) -> p m two", two=2)
                    nc.vector.tensor_copy(
                        f_next[0:64, b * M_out : (b + 1) * M_out]
                        .bitcast(f32r)
                        .rearrange("p (m o) -> p m o", o=1),
                        ev[:, :, 0:1],
                    )
                    nc.vector.tensor_copy(
                        f_next[64:128, b * M_out : (b + 1) * M_out]
                        .bitcast(f32r)
                        .rearrange("p (m o) -> p m o", o=1),
                        od[:, :, 1:2],
                    )
                else:
                    h2d = h2pool.tile([128, 512], f32, tag="h2", name=f"h2_{i}_{b}")
                    nc.scalar.activation(
                        h2d[0:64, 0:C],
                        w2t[0:64, off : off + C],
                        AF.Relu,
                        bias=b2d_sb[i][0:64, :],
                    )
                nc.vector.tensor_reduce(
                    reps[:, 2 * i + b : 2 * i + b + 1],
                    h2d[0:64, 0:C],
                    axis=AX.X,
                    op=ALU.add,
                )
            if i < 4:
                f_prev = f_next

        # ---- tail: mult, feats, logit ----
        Lv = L_all[0:1, 0:10].rearrange("p (i b) -> p b i", b=2)
        mred = smallp.tile([1, 2], f32, tag="mred", name="mred")
        nc.vector.tensor_reduce(mred[:], Lv, axis=AX.X, op=ALU.add)
        mult = smallp.tile([1, 2], f32, tag="mult", name="mult")
        nc.scalar.activation(mult[:], mred[:], AF.Identity, bias=1.0, scale=0.5)

        featsA = constp.tile([128, 2], f32, tag="fA", name="featsA")
        featsB = constp.tile([128, 2], f32, tag="fB", name="featsB")
        featsC = constp.tile([65, 2], f32, tag="fC", name="featsC")
        nc.sync.dma_start(featsA[0:64, :], reps[:, 0:2])
        nc.sync.dma_start(featsA[64:128, :], reps[:, 2:4])
        nc.sync.dma_start(featsB[0:64, :], reps[:, 4:6])
        nc.sync.dma_start(featsB[64:128, :], reps[:, 6:8])
        nc.sync.dma_start(featsC[0:64, :], reps[:, 8:10])
        nc.sync.dma_start(featsC[64:65, :], mult[:])

        wrA = constp.tile([128, 1], f32, tag="wrA", name="wrA")
        wrB = constp.tile([128, 1], f32, tag="wrB", name="wrB")
        wrC = constp.tile([65, 1], f32, tag="wrC", name="wrC")
        nc.sync.dma_start(wrA[:], wr_dram[0:128, :])
        nc.sync.dma_start(wrB[:], wr_dram[128:256, :])
        nc.sync.dma_start(wrC[:], wr_dram[256:321, :])
        brt = constp.tile([1, 1], f32, tag="br", name="brt")
        nc.sync.dma_start(brt[:], br_dram[:])

        logps = psump.tile([1, 512], f32, tag="small", name="logps")
        nc.tensor.matmul(logps[0:1, 0:2], wrA[:], featsA[:], start=True, stop=False)
        nc.tensor.matmul(logps[0:1, 0:2], wrB[:], featsB[:], start=False, stop=False)
        nc.tensor.matmul(logps[0:1, 0:2], wrC[:], featsC[:], start=False, stop=True)
        outsb = smallp.tile([1, 2], f32, tag="outsb", name="outsb")
        nc.scalar.activation(
            outsb[:], logps[0:1, 0:2], AF.Identity, bias=brt[:], scale=1.0
        )
        nc.sync.dma_start(out_dram[:, 0:1], outsb[0:1, :])

    nc.compile()
    return nc


def _get_nc():
    if "nc" not in _CACHE:
        _CACHE["nc"] = _build()
    return _CACHE["nc"]


def make_in_maps(inputs):
    in_maps = []
    for core in range(N_CORES):
        s = core * B_LOC
        m = {}
        for i in range(5):
            m[f"p{i}"] = np.ascontiguousarray(inputs[f"p{i}"][s : s + B_LOC])
            m[f"W1_{i}"] = inputs[f"W1_{i}"]
            m[f"b1_{i}"] = inputs[f"b1_{i}"]
            m[f"W2_{i}"] = inputs[f"W2_{i}"]
            m[f"b2_{i}"] = inputs[f"b2_{i}"]
        m["Wr"] = inputs["Wr"]
        m["br"] = inputs["br"]
        in_maps.append(m)
    return in_maps


def run(inputs, trace=False):
    from concourse.bass_utils import run_bass_kernel_spmd

    nc = _get_nc()
    in_maps = make_in_maps(inputs)
    res = run_bass_kernel_spmd(nc, in_maps, list(range(N_CORES)), trace=trace)
    out = np.concatenate([r["out"] for r in res.results], axis=0)
    return out.astype(np.float32), res


def kernel(**inputs):
    inputs = {k: np.asarray(v, dtype=np.float32) for k, v in inputs.items()}
    out, _ = run(inputs, trace=False)
    return out


# revision 32
# speedup vs baseline: 1.0536x; 1.0536x over previous
import sys

sys.path.insert(0, "/opt/trn_rl_repo")

import numpy as np

B = 16
N_CORES = 8
B_LOC = B // N_CORES  # 2 samples per core
H1, H2 = 128, 64
DIM_CH = [3, 64, 128, 256, 512]
CS = [64, 128, 256, 512, 512]
FS = [(1 + c) * 9 for c in DIM_CH]  # [36, 585, 1161, 2313, 4617]
DINS = [FS[0]] + [FS[i] + CS[i - 1] * H2 for i in range(1, 5)]
NCH = [(f + 127) // 128 for f in FS]  # [1, 5, 10, 19, 37]
BASE = [2 * sum(NCH[:i]) for i in range(5)]
TOT_COLS = 2 * sum(NCH)  # 144

_CACHE = {}


def _build():
    from contextlib import ExitStack

    import concourse.bacc as bacc
    import concourse.tile as tile
    from concourse import mybir

    f32 = mybir.dt.float32
    f32r = mybir.dt.float32r
    AF = mybir.ActivationFunctionType
    ALU = mybir.AluOpType
    AX = mybir.AxisListType

    nc = bacc.Bacc(None, target_bir_lowering=False)

    p_dram = [
        nc.dram_tensor(f"p{i}", [B_LOC, FS[i], CS[i]], f32, kind="ExternalInput")
        for i in range(5)
    ]
    w1_dram = [
        nc.dram_tensor(f"W1_{i}", [DINS[i], H1], f32, kind="ExternalInput")
        for i in range(5)
    ]
    b1_dram = [
        nc.dram_tensor(f"b1_{i}", [H1], f32, kind="ExternalInput") for i in range(5)
    ]
    w2_dram = [
        nc.dram_tensor(f"W2_{i}", [H1, H2], f32, kind="ExternalInput")
        for i in range(5)
    ]
    b2_dram = [
        nc.dram_tensor(f"b2_{i}", [H2], f32, kind="ExternalInput") for i in range(5)
    ]
    wr_dram = nc.dram_tensor("Wr", [5 * H2 + 1, 1], f32, kind="ExternalInput")
    br_dram = nc.dram_tensor("br", [1], f32, kind="ExternalInput")
    out_dram = nc.dram_tensor("out", [B_LOC, 1], f32, kind="ExternalOutput")

    from concourse import masks as cmasks

    with tile.TileContext(nc) as tc, ExitStack() as ctx:
        constp = ctx.enter_context(tc.tile_pool(name="const", bufs=1))
        xpool = ctx.enter_context(tc.tile_pool(name="x", bufs=8))
        wfpool = ctx.enter_context(tc.tile_pool(name="wf", bufs=4))
        wppool = ctx.enter_context(tc.tile_pool(name="wp", bufs=6))
        hpool = ctx.enter_context(tc.tile_pool(name="h", bufs=3))
        h2pool = ctx.enter_context(tc.tile_pool(name="h2", bufs=3))
        fpool = ctx.enter_context(tc.tile_pool(name="f", bufs=2))
        smallp = ctx.enter_context(tc.tile_pool(name="small", bufs=2))
        psump = ctx.enter_context(tc.tile_pool(name="ps", bufs=1, space="PSUM"))

        ones_col = constp.tile([128, 1], f32, tag="ones_col", name="ones_col")
        nc.vector.memset(ones_col[:], 1.0)
        ones_row = constp.tile([1, 128], f32, tag="ones_row", name="ones_row")
        nc.vector.memset(ones_row[:], 1.0)
        stats = constp.tile([128, TOT_COLS], f32, tag="stats", name="stats")
        nc.vector.memset(stats[:], 0.0)
        reps = constp.tile([64, 10], f32, tag="reps", name="reps")
        L_all = constp.tile([1, 10], f32, tag="L_all", name="L_all")
        sqscr = constp.tile([128, 512], f32, tag="sqscr", name="sqscr")
        id8 = constp.tile([8, 8], f32, tag="id8", name="id8")
        cmasks.make_identity(nc, id8[:])

        b1_sb, b2d_sb, w2d_sb = [], [], []
        for i in range(5):
            b1t = constp.tile([H1, 1], f32, tag=f"b1_{i}", name=f"b1s_{i}")
            nc.sync.dma_start(b1t[:], b1_dram[i][:])
            b1_sb.append(b1t)
            b2t = constp.tile([128, 1], f32, tag=f"b2_{i}", name=f"b2s_{i}")
            nc.sync.dma_start(b2t[0:64, :], b2_dram[i][:])
            nc.sync.dma_start(b2t[64:128, :], b2_dram[i][:])
            b2d_sb.append(b2t)
            w2t = constp.tile([H1, 128], f32, tag=f"w2_{i}", name=f"w2s_{i}")
            nc.sync.dma_start(w2t[:, 0:64].bitcast(f32r), w2_dram[i][:].bitcast(f32r))
            nc.sync.dma_start(w2t[:, 64:128].bitcast(f32r), w2_dram[i][:].bitcast(f32r))
            w2d_sb.append(w2t)

        f_prev = None
        for i in range(5):
            C, F, nch = CS[i], FS[i], NCH[i]
            packed = 2 * C <= 512

            # ---- prev-features matmuls (i > 0): prevT = W1p.T @ f_prev ----
            prevps = None
            if i > 0:
                M_in = CS[i - 1] * H2 // 128
                nt = M_in // 4
                prevG = psump.tile(
                    [128, 512], f32, tag="prev", bufs=1, name=f"prevG_{i}"
                )
                for t in range(nt):
                    wp4 = wppool.tile([128, 512], f32, tag="wp4", name=f"wp4_{i}_{t}")
                    nc.sync.dma_start(
                        wp4[:].bitcast(f32r).rearrange("p (j h) -> p j h", j=4),
                        w1_dram[i][F + t * 512 : F + (t + 1) * 512, :]
                        .bitcast(f32r)
                        .rearrange("(j p) h -> p j h", j=4),
                    )
                    nc.tensor.matmul(
                        prevG[0:8, 0:512],
                        f_prev[:, 8 * t : 8 * t + 8].bitcast(f32r),
                        wp4[:].bitcast(f32r),
                        start=(t == 0),
                        stop=(t == nt - 1),
                    )
                # real data sits in diagonal [2,128] blocks of prevG:
                # transpose each [8,128] block, then strided-reduce diagonals
                pg = smallp.tile([8, 512], f32, tag="pg", name=f"pg_{i}")
                nc.vector.tensor_copy(pg[:], prevG[0:8, 0:512])
                tpA = psump.tile([128, 40], f32, tag="prevT", bufs=1, name=f"tpA_{i}")
                for j in range(4):
                    nc.tensor.transpose(
                        tpA[0:128, 8 * j : 8 * j + 8],
                        pg[0:8, j * 128 : (j + 1) * 128],
                        id8[:],
                    )
                prevps = smallp.tile([128, 2], f32, tag="pvs", name=f"pvs_{i}")
                # block j's diagonal sits at column 8j + (2j+b) = 10j + b
                tpv = tpA[0:128, 0:40].rearrange("p (g r) -> p r g", r=10)
                for b in range(2):
                    nc.vector.tensor_reduce(
                        prevps[:, b : b + 1],
                        tpv[:, b : b + 1, :],
                        axis=AX.X,
                        op=ALU.add,
                    )

            # ---- z matmuls (accumulate over F chunks) + sumsq stats ----
            if packed:
                zps = psump.tile([128, 512], f32, tag="z", bufs=3, name=f"zps_{i}")
                for k in range(nch):
                    kl = min(128, F - k * 128)
                    xt = xpool.tile([128, 2 * C], f32, tag="x", name=f"x_{i}_{k}")
                    for b in range(2):
                        nc.sync.dma_start(
                            xt[0:kl, b * C : (b + 1) * C].bitcast(f32r),
                            p_dram[i][b, k * 128 : k * 128 + kl, :].bitcast(f32r),
                        )
                    wf = wfpool.tile([128, 128], f32, tag="wf", name=f"wf_{i}_{k}")
                    nc.sync.dma_start(
                        wf[0:kl, :].bitcast(f32r),
                        w1_dram[i][k * 128 : k * 128 + kl, :].bitcast(f32r),
                    )
                    nc.tensor.matmul(
                        zps[0:128, 0 : 2 * C],
                        wf[0:kl, :].bitcast(f32r),
                        xt[0:kl, :].bitcast(f32r),
                        start=(k == 0),
                        stop=(k == nch - 1),
                    )
                    for b in range(2):
                        col = BASE[i] + b * nch + k
                        nc.scalar.activation(
                            sqscr[0:kl, 0:C],
                            xt[0:kl, b * C : (b + 1) * C],
                            AF.Square,
                            accum_out=stats[0:kl, col : col + 1],
                        )
                z_list = [(zps, 0), (zps, C)]
            else:
                zpsA = psump.tile([128, 512], f32, tag="z", bufs=3, name=f"zpsA_{i}")
                zpsB = psump.tile([128, 512], f32, tag="z", bufs=3, name=f"zpsB_{i}")
                for k in range(nch):
                    kl = min(128, F - k * 128)
                    wf = wfpool.tile([128, 128], f32, tag="wf", name=f"wf_{i}_{k}")
                    nc.sync.dma_start(
                        wf[0:kl, :].bitcast(f32r),
                        w1_dram[i][k * 128 : k * 128 + kl, :].bitcast(f32r),
                    )
                    for b, zp in ((0, zpsA), (1, zpsB)):
                        xt = xpool.tile([128, 512], f32, tag="x", name=f"x_{i}_{k}_{b}")
                        nc.sync.dma_start(
                            xt[0:kl, 0:C].bitcast(f32r),
                            p_dram[i][b, k * 128 : k * 128 + kl, :].bitcast(f32r),
                        )
                        nc.tensor.matmul(
                            zp[0:128, 0:C],
                            wf[0:kl, :].bitcast(f32r),
                            xt[0:kl, 0:C].bitcast(f32r),
                            start=(k == 0),
                            stop=(k == nch - 1),
                        )
                        col = BASE[i] + b * nch + k
                        nc.scalar.activation(
                            sqscr[0:kl, 0:C],
                            xt[0:kl, 0:C],
                            AF.Square,
                            accum_out=stats[0:kl, col : col + 1],
                        )
                z_list = [(zpsA, 0), (zpsB, 0)]

            # ---- scale factors: sumsq -> ln -> exp(-0.5 ln) broadcast ----
            st2 = smallp.tile([128, 2], f32, tag="st2", name=f"st2_{i}")
            for b in range(2):
                nc.vector.tensor_reduce(
                    st2[:, b : b + 1],
                    stats[:, BASE[i] + b * nch : BASE[i] + (b + 1) * nch],
                    axis=AX.X,
                    op=ALU.add,
                )
            sumps = psump.tile([1, 512], f32, tag="small", name=f"sumps_{i}")
            nc.tensor.matmul(
                sumps[0:1, 0:2], ones_col[:], st2[:], start=True, stop=True
            )
            nc.scalar.activation(L_all[0:1, 2 * i : 2 * i + 2], sumps[0:1, 0:2], AF.Ln)
            inv_sf = smallp.tile([1, 2], f32, tag="invsf", name=f"invsf_{i}")
            nc.scalar.activation(
                inv_sf[:], L_all[0:1, 2 * i : 2 * i + 2], AF.Exp, scale=-0.5
            )
            bcps = psump.tile([128, 512], f32, tag="small", name=f"bcps_{i}")
            nc.tensor.matmul(
                bcps[0:128, 0:2], ones_row[:], inv_sf[:], start=True, stop=True
            )
            inv_bc = smallp.tile([128, 2], f32, tag="invbc", name=f"invbc_{i}")
            nc.vector.tensor_copy(inv_bc[:], bcps[0:128, 0:2])

            # ---- bias = b1 (+ prevT) ----
            if i > 0:
                btot = smallp.tile([128, 2], f32, tag="btot", name=f"btot_{i}")
                nc.vector.tensor_scalar_add(btot[:], prevps[0:128, 0:2], b1_sb[i][:])
                bias_aps = [btot[:, 0:1], btot[:, 1:2]]
            else:
                bias_aps = [b1_sb[0][:], b1_sb[0][:]]

            # ---- h = relu(z/sf + bias) ----
            if packed:
                ht = hpool.tile([128, 2 * C], f32, tag="h", name=f"ht_{i}")
                for b in range(2):
                    nc.scalar.activation(
                        ht[:, b * C : (b + 1) * C].bitcast(f32r),
                        zps[0:128, b * C : (b + 1) * C],
                        AF.Relu,
                        bias=bias_aps[b],
                        scale=inv_bc[:, b : b + 1],
                    )
                h_list = [(ht, 0), (ht, C)]
            else:
                htA = hpool.tile([128, 512], f32, tag="h", name=f"htA_{i}")
                htB = hpool.tile([128, 512], f32, tag="h", name=f"htB_{i}")
                for b, h in ((0, htA), (1, htB)):
                    zp, off = z_list[b]
                    nc.scalar.activation(
                        h[:, 0:C].bitcast(f32r),
                        zp[0:128, off : off + C],
                        AF.Relu,
                        bias=bias_aps[b],
                        scale=inv_bc[:, b : b + 1],
                    )
                h_list = [(htA, 0), (htB, 0)]

            # ---- h2 = relu(h @ W2 + b2), with duplicated W2 for fold ----
            if packed:
                w2ps = psump.tile([128, 512], f32, tag="w2", bufs=2, name=f"w2ps_{i}")
                nc.tensor.matmul(
                    w2ps[0:128, 0 : 2 * C],
                    w2d_sb[i][:].bitcast(f32r),
                    ht[:, 0 : 2 * C].bitcast(f32r),
                    start=True,
                    stop=True,
                )
                w2_list = [(w2ps, 0), (w2ps, C)]
            else:
                w2psA = psump.tile([128, 512], f32, tag="w2", bufs=2, name=f"w2psA_{i}")
                w2psB = psump.tile([128, 512], f32, tag="w2", bufs=2, name=f"w2psB_{i}")
                for b, wp in ((0, w2psA), (1, w2psB)):
                    h, off = h_list[b]
                    nc.tensor.matmul(
                        wp[0:128, 0:C],
                        w2d_sb[i][:].bitcast(f32r),
                        h[:, off : off + C].bitcast(f32r),
                        start=True,
                        stop=True,
                    )
                w2_list = [(w2psA, 0), (w2psB, 0)]

            # ---- relu + reps + fold into next-layer prev features ----
            f_next = None
            if i < 4:
                M_out = C // 2
                f_next = fpool.tile([128, 2 * M_out], f32, tag="f", name=f"f_{i}")
            for b in range(2):
                w2t, off = w2_list[b]
                if i < 4:
                    h2d = h2pool.tile([128, 512], f32, tag="h2", name=f"h2_{i}_{b}")
                    nc.scalar.activation(
                        h2d[:, 0:C],
                        w2t[0:128, off : off + C],
                        AF.Relu,
                        bias=b2d_sb[i][:],
                    )
                    ev = h2d[0:64, 0:C].rearrange("p (m two) -> p m two", two=2)
                    od = h2d[64:128, 0:C].rearrange("p (m two) -> p m two", two=2)
                    # interleaved layout: f[:, 2*m + b] = prev chunk m, sample b
                    fe = (
                        f_next[0:64, 0 : 2 * M_out]
                        .bitcast(f32r)
                        .rearrange("p (m b) -> p m b", b=2)
                    )
                    fo = (
                        f_next[64:128, 0 : 2 * M_out]
                        .bitcast(f32r)
                        .rearrange("p (m b) -> p m b", b=2)
                    )
                    nc.vector.tensor_copy(fe[:, :, b : b + 1], ev[:, :, 0:1])
                    nc.vector.tensor_copy(fo[:, :, b : b + 1], od[:, :, 1:2])
                else:
                    h2d = h2pool.tile([128, 512], f32, tag="h2", name=f"h2_{i}_{b}")
                    nc.scalar.activation(
                        h2d[0:64, 0:C],
                        w2t[0:64, off : off + C],
                        AF.Relu,
                        bias=b2d_sb[i][0:64, :],
                    )
                nc.vector.tensor_reduce(
                    reps[:, 2 * i + b : 2 * i + b + 1],
                    h2d[0:64, 0:C],
                    axis=AX.X,
                    op=ALU.add,
                )
            if i < 4:
                f_prev = f_next

        # ---- tail: mult, feats, logit ----
        Lv = L_all[0:1, 0:10].rearrange("p (i b) -> p b i", b=2)
        mred = smallp.tile([1, 2], f32, tag="mred", name="mred")
        nc.vector.tensor_reduce(mred[:], Lv, axis=AX.X, op=ALU.add)
        mult = smallp.tile([1, 2], f32, tag="mult", name="mult")
        nc.scalar.activation(mult[:], mred[:], AF.Identity, bias=1.0, scale=0.5)

        featsA = constp.tile([128, 2], f32, tag="fA", name="featsA")
        featsB = constp.tile([128, 2], f32, tag="fB", name="featsB")
        featsC = constp.tile([65, 2], f32, tag="fC", name="featsC")
        nc.sync.dma_start(featsA[0:64, :], reps[:, 0:2])
        nc.sync.dma_start(featsA[64:128, :], reps[:, 2:4])
        nc.sync.dma_start(featsB[0:64, :], reps[:, 4:6])
        nc.sync.dma_start(featsB[64:128, :], reps[:, 6:8])
        nc.sync.dma_start(featsC[0:64, :], reps[:, 8:10])
        nc.sync.dma_start(featsC[64:65, :], mult[:])

        wrA = constp.tile([128, 1], f32, tag="wrA", name="wrA")
        wrB = constp.tile([128, 1], f32, tag="wrB", name="wrB")
        wrC = constp.tile([65, 1], f32, tag="wrC", name="wrC")
        nc.sync.dma_start(wrA[:], wr_dram[0:128, :])
        nc.sync.dma_start(wrB[:], wr_dram[128:256, :])
        nc.sync.dma_start(wrC[:], wr_dram[256:321, :])
        brt = constp.tile([1, 1], f32, tag="br", name="brt")
        nc.sync.dma_start(brt[:], br_dram[:])

        logps = psump.tile([1, 512], f32, tag="small", name="logps")
        nc.tensor.matmul(logps[0:1, 0:2], wrA[:], featsA[:], start=True, stop=False)
        nc.tensor.matmul(logps[0:1, 0:2], wrB[:], featsB[:], start=False, stop=False)
        nc.tensor.matmul(logps[0:1, 0:2], wrC[:], featsC[:], start=False, stop=True)
        outsb = smallp.tile([1, 2], f32, tag="outsb", name="outsb")
        nc.scalar.activation(
            outsb[:], logps[0:1, 0:2], AF.Identity, bias=brt[:], scale=1.0
        )
        nc.sync.dma_start(out_dram[:, 0:1], outsb[0:1, :])

    nc.compile()
    return nc


def _get_nc():
    if "nc" not in _CACHE:
        _CACHE["nc"] = _build()
    return _CACHE["nc"]


def make_in_maps(inputs):
    in_maps = []
    for core in range(N_CORES):
        s = core * B_LOC
        m = {}
        for i in range(5):
            m[f"p{i}"] = np.ascontiguousarray(inputs[f"p{i}"][s : s + B_LOC])
            m[f"W1_{i}"] = inputs[f"W1_{i}"]
            m[f"b1_{i}"] = inputs[f"b1_{i}"]
            m[f"W2_{i}"] = inputs[f"W2_{i}"]
            m[f"b2_{i}"] = inputs[f"b2_{i}"]
        m["Wr"] = inputs["Wr"]
        m["br"] = inputs["br"]
        in_maps.append(m)
    return in_maps


def run(inputs, trace=False):
    from concourse.bass_utils import run_bass_kernel_spmd

    nc = _get_nc()
    in_maps = make_in_maps(inputs)
    res = run_bass_kernel_spmd(nc, in_maps, list(range(N_CORES)), trace=trace)
    out = np.concatenate([r["out"] for r in res.results], axis=0)
    return out.astype(np.float32), res


def kernel(**inputs):
    inputs = {k: np.asarray(v, dtype=np.float32) for k, v in inputs.items()}
    out, _ = run(inputs, trace=False)
    return out


# revision 41
# speedup vs baseline: 1.6002x; 1.5189x over previous
import sys

sys.path.insert(0, "/opt/trn_rl_repo")

import numpy as np

B = 16
N_CORES = 8
B_LOC = B // N_CORES  # 2 samples per core
H1, H2 = 128, 64
DIM_CH = [3, 64, 128, 256, 512]
CS = [64, 128, 256, 512, 512]
FS = [(1 + c) * 9 for c in DIM_CH]  # [36, 585, 1161, 2313, 4617]
DINS = [FS[0]] + [FS[i] + CS[i - 1] * H2 for i in range(1, 5)]
NCH = [(f + 127) // 128 for f in FS]  # [1, 5, 10, 19, 37]
BASE = [2 * sum(NCH[:i]) for i in range(5)]
TOT_COLS = 2 * sum(NCH)  # 144

_CACHE = {}


def _build():
    from contextlib import ExitStack

    import concourse.bacc as bacc
    import concourse.tile as tile
    from concourse import mybir

    f32 = mybir.dt.float32
    f32r = mybir.dt.float32r
    bf16 = mybir.dt.bfloat16
    AF = mybir.ActivationFunctionType
    ALU = mybir.AluOpType
    AX = mybir.AxisListType

    nc = bacc.Bacc(None, target_bir_lowering=False)

    p_dram = [
        nc.dram_tensor(f"p{i}", [B_LOC, FS[i], CS[i]], bf16, kind="ExternalInput")
        for i in range(5)
    ]
    w1_dram = [
        nc.dram_tensor(f"W1_{i}", [DINS[i], H1], bf16, kind="ExternalInput")
        for i in range(5)
    ]
    b1_dram = [
        nc.dram_tensor(f"b1_{i}", [H1], f32, kind="ExternalInput") for i in range(5)
    ]
    w2_dram = [
        nc.dram_tensor(f"W2_{i}", [H1, H2], f32, kind="ExternalInput")
        for i in range(5)
    ]
    b2_dram = [
        nc.dram_tensor(f"b2_{i}", [H2], f32, kind="ExternalInput") for i in range(5)
    ]
    wr_dram = nc.dram_tensor("Wr", [5 * H2 + 1, 1], f32, kind="ExternalInput")
    br_dram = nc.dram_tensor("br", [1], f32, kind="ExternalInput")
    out_dram = nc.dram_tensor("out", [B_LOC, 1], f32, kind="ExternalOutput")

    from concourse import masks as cmasks

    with tile.TileContext(nc) as tc, ExitStack() as ctx:
        constp = ctx.enter_context(tc.tile_pool(name="const", bufs=1))
        xpool = ctx.enter_context(tc.tile_pool(name="x", bufs=8))
        wfpool = ctx.enter_context(tc.tile_pool(name="wf", bufs=4))
        wppool = ctx.enter_context(tc.tile_pool(name="wp", bufs=6))
        hpool = ctx.enter_context(tc.tile_pool(name="h", bufs=3))
        h2pool = ctx.enter_context(tc.tile_pool(name="h2", bufs=3))
        fpool = ctx.enter_context(tc.tile_pool(name="f", bufs=2))
        smallp = ctx.enter_context(tc.tile_pool(name="small", bufs=2))
        psump = ctx.enter_context(tc.tile_pool(name="ps", bufs=1, space="PSUM"))

        ones_col = constp.tile([128, 1], f32, tag="ones_col", name="ones_col")
        nc.vector.memset(ones_col[:], 1.0)
        ones_row = constp.tile([1, 128], f32, tag="ones_row", name="ones_row")
        nc.vector.memset(ones_row[:], 1.0)
        stats = constp.tile([128, TOT_COLS], f32, tag="stats", name="stats")
        nc.vector.memset(stats[:], 0.0)
        reps = constp.tile([64, 10], f32, tag="reps", name="reps")
        L_all = constp.tile([1, 10], f32, tag="L_all", name="L_all")
        sqscr = constp.tile([128, 512], f32, tag="sqscr", name="sqscr")
        id8 = constp.tile([8, 8], f32, tag="id8", name="id8")
        cmasks.make_identity(nc, id8[:])

        b1_sb, b2d_sb, w2d_sb = [], [], []
        for i in range(5):
            b1t = constp.tile([H1, 1], f32, tag=f"b1_{i}", name=f"b1s_{i}")
            nc.gpsimd.dma_start(b1t[:], b1_dram[i][:])
            b1_sb.append(b1t)
            b2t = constp.tile([128, 1], f32, tag=f"b2_{i}", name=f"b2s_{i}")
            nc.gpsimd.dma_start(b2t[0:64, :], b2_dram[i][:])
            nc.gpsimd.dma_start(b2t[64:128, :], b2_dram[i][:])
            b2d_sb.append(b2t)
            w2t = constp.tile([H1, 128], f32, tag=f"w2_{i}", name=f"w2s_{i}")
            nc.gpsimd.dma_start(w2t[:, 0:64].bitcast(f32r), w2_dram[i][:].bitcast(f32r))
            nc.gpsimd.dma_start(
                w2t[:, 64:128].bitcast(f32r), w2_dram[i][:].bitcast(f32r)
            )
            w2d_sb.append(w2t)

        f_prev = None
        for i in range(5):
            C, F, nch = CS[i], FS[i], NCH[i]
            packed = 2 * C <= 512

            # ---- prev-features matmuls (i > 0): prevT = W1p.T @ f_prev ----
            prevps = None
            if i > 0:
                D = CS[i - 1] * H2
                ntb = D // 2048
                prevG = psump.tile(
                    [128, 512], f32, tag="prev", bufs=1, name=f"prevG_{i}"
                )
                for t in range(ntb):
                    wpB = wppool.tile([128, 2048], bf16, tag="wp4", name=f"wp_{i}_{t}")
                    nc.sync.dma_start(
                        wpB[:].rearrange("p (j h) -> p j h", j=16),
                        w1_dram[i][F + t * 2048 : F + (t + 1) * 2048, :].rearrange(
                            "(j p) h -> p j h", j=16
                        ),
                    )
                    for q in range(4):
                        nc.tensor.matmul(
                            prevG[0:8, 0:512],
                            f_prev[:, 32 * t + 8 * q : 32 * t + 8 * q + 8],
                            wpB[:, 512 * q : 512 * (q + 1)],
                            start=(t == 0 and q == 0),
                            stop=(t == ntb - 1 and q == 3),
                        )
                # real data sits in diagonal [2,128] blocks of prevG:
                # transpose each [8,128] block, then strided-reduce diagonals
                pg = smallp.tile([8, 512], f32, tag="pg", name=f"pg_{i}")
                nc.vector.tensor_copy(pg[:], prevG[0:8, 0:512])
                tpA = psump.tile([128, 40], f32, tag="prevT", bufs=1, name=f"tpA_{i}")
                for j in range(4):
                    nc.tensor.transpose(
                        tpA[0:128, 8 * j : 8 * j + 8],
                        pg[0:8, j * 128 : (j + 1) * 128],
                        id8[:],
                    )
                prevps = smallp.tile([128, 2], f32, tag="pvs", name=f"pvs_{i}")
                # block j's diagonal sits at column 8j + (2j+b) = 10j + b
                tpv = tpA[0:128, 0:40].rearrange("p (g r) -> p r g", r=10)
                for b in range(2):
                    nc.vector.tensor_reduce(
                        prevps[:, b : b + 1],
                        tpv[:, b : b + 1, :],
                        axis=AX.X,
                        op=ALU.add,
                    )

            # ---- batched loads of x and W1f (512-row blocks) ----
            nb = F // 512
            rem = F - nb * 512
            nfull = rem // 128
            last = rem - nfull * 128
            jjs = [4] * nb + ([nfull] if nfull else [])
            chunk_x = []  # (tile, base_col, kl); sample b cols = base + b*C
            chunk_wf = []  # (tile, col, kl)
            r = 0
            for bi, jj in enumerate(jjs):
                xt = xpool.tile([128, jj * 2 * C], bf16, tag="x", name=f"x_{i}_{bi}")
                xv = xt[:].rearrange("p (j bc) -> p j bc", j=jj)
                for b in range(2):
                    nc.sync.dma_start(
                        xv[:, :, b * C : (b + 1) * C],
                        p_dram[i][b, r : r + jj * 128, :].rearrange(
                            "(j f) c -> f j c", j=jj
                        ),
                    )
                wft = wfpool.tile([128, jj * 128], bf16, tag="wf", name=f"wf_{i}_{bi}")
                nc.sync.dma_start(
                    wft[:].rearrange("p (j h) -> p j h", j=jj),
                    w1_dram[i][r : r + jj * 128, :].rearrange("(j p) h -> p j h", j=jj),
                )
                for j in range(jj):
                    chunk_x.append((xt, j * 2 * C, 128))
                    chunk_wf.append((wft, j * 128, 128))
                r += jj * 128
            if last:
                xt = xpool.tile([128, 2 * C], bf16, tag="x", name=f"x_{i}_last")
                nc.sync.dma_start(
                    xt[0:last, :].rearrange("p (b c) -> p b c", b=2),
                    p_dram[i][:, r:F, :].rearrange("b f c -> f b c"),
                )
                wft = wfpool.tile([128, 128], bf16, tag="wf", name=f"wf_{i}_last")
                nc.sync.dma_start(wft[0:last, :], w1_dram[i][r:F, :])
                chunk_x.append((xt, 0, last))
                chunk_wf.append((wft, 0, last))

            # ---- z matmuls (accumulate over F chunks) + sumsq stats ----
            if packed:
                zps = psump.tile([128, 512], f32, tag="z", bufs=3, name=f"zps_{i}")
                for k in range(nch):
                    xt, xb, kl = chunk_x[k]
                    wft, wc, _ = chunk_wf[k]
                    nc.tensor.matmul(
                        zps[0:128, 0 : 2 * C],
                        wft[0:kl, wc : wc + 128],
                        xt[0:kl, xb : xb + 2 * C],
                        start=(k == 0),
                        stop=(k == nch - 1),
                    )
                    for b in range(2):
                        col = BASE[i] + b * nch + k
                        nc.scalar.activation(
                            sqscr[0:kl, 0:C],
                            xt[0:kl, xb + b * C : xb + (b + 1) * C],
                            AF.Square,
                            accum_out=stats[0:kl, col : col + 1],
                        )
                z_list = [(zps, 0), (zps, C)]
            else:
                zpsA = psump.tile([128, 512], f32, tag="z", bufs=3, name=f"zpsA_{i}")
                zpsB = psump.tile([128, 512], f32, tag="z", bufs=3, name=f"zpsB_{i}")
                for k in range(nch):
                    xt, xb, kl = chunk_x[k]
                    wft, wc, _ = chunk_wf[k]
                    for b, zp in ((0, zpsA), (1, zpsB)):
                        nc.tensor.matmul(
                            zp[0:128, 0:C],
                            wft[0:kl, wc : wc + 128],
                            xt[0:kl, xb + b * C : xb + (b + 1) * C],
                            start=(k == 0),
                            stop=(k == nch - 1),
                        )
                        col = BASE[i] + b * nch + k
                        nc.scalar.activation(
                            sqscr[0:kl, 0:C],
                            xt[0:kl, xb + b * C : xb + (b + 1) * C],
                            AF.Square,
                            accum_out=stats[0:kl, col : col + 1],
                        )
                z_list = [(zpsA, 0), (zpsB, 0)]

            # ---- scale factors: sumsq -> ln -> exp(-0.5 ln) broadcast ----
            st2 = smallp.tile([128, 2], f32, tag="st2", name=f"st2_{i}")
            for b in range(2):
                nc.vector.tensor_reduce(
                    st2[:, b : b + 1],
                    stats[:, BASE[i] + b * nch : BASE[i] + (b + 1) * nch],
                    axis=AX.X,
                    op=ALU.add,
                )
            sumps = psump.tile([1, 512], f32, tag="small", name=f"sumps_{i}")
            nc.tensor.matmul(
                sumps[0:1, 0:2], ones_col[:], st2[:], start=True, stop=True
            )
            nc.scalar.activation(L_all[0:1, 2 * i : 2 * i + 2], sumps[0:1, 0:2], AF.Ln)
            inv_sf = smallp.tile([1, 2], f32, tag="invsf", name=f"invsf_{i}")
            nc.scalar.activation(
                inv_sf[:], L_all[0:1, 2 * i : 2 * i + 2], AF.Exp, scale=-0.5
            )
            bcps = psump.tile([128, 512], f32, tag="small", name=f"bcps_{i}")
            nc.tensor.matmul(
                bcps[0:128, 0:2], ones_row[:], inv_sf[:], start=True, stop=True
            )
            inv_bc = smallp.tile([128, 2], f32, tag="invbc", name=f"invbc_{i}")
            nc.vector.tensor_copy(inv_bc[:], bcps[0:128, 0:2])

            # ---- bias = b1 (+ prevT) ----
            if i > 0:
                btot = smallp.tile([128, 2], f32, tag="btot", name=f"btot_{i}")
                nc.vector.tensor_scalar_add(btot[:], prevps[0:128, 0:2], b1_sb[i][:])
                bias_aps = [btot[:, 0:1], btot[:, 1:2]]
            else:
                bias_aps = [b1_sb[0][:], b1_sb[0][:]]

            # ---- h = relu(z/sf + bias) ----
            if packed:
                ht = hpool.tile([128, 2 * C], f32, tag="h", name=f"ht_{i}")
                for b in range(2):
                    nc.scalar.activation(
                        ht[:, b * C : (b + 1) * C].bitcast(f32r),
                        zps[0:128, b * C : (b + 1) * C],
                        AF.Relu,
                        bias=bias_aps[b],
                        scale=inv_bc[:, b : b + 1],
                    )
                h_list = [(ht, 0), (ht, C)]
            else:
                htA = hpool.tile([128, 512], f32, tag="h", name=f"htA_{i}")
                htB = hpool.tile([128, 512], f32, tag="h", name=f"htB_{i}")
                for b, h in ((0, htA), (1, htB)):
                    zp, off = z_list[b]
                    nc.scalar.activation(
                        h[:, 0:C].bitcast(f32r),
                        zp[0:128, off : off + C],
                        AF.Relu,
                        bias=bias_aps[b],
                        scale=inv_bc[:, b : b + 1],
                    )
                h_list = [(htA, 0), (htB, 0)]

            # ---- h2 = relu(h @ W2 + b2), with duplicated W2 for fold ----
            if packed:
                w2ps = psump.tile([128, 512], f32, tag="w2", bufs=2, name=f"w2ps_{i}")
                nc.tensor.matmul(
                    w2ps[0:128, 0 : 2 * C],
                    w2d_sb[i][:].bitcast(f32r),
                    ht[:, 0 : 2 * C].bitcast(f32r),
                    start=True,
                    stop=True,
                )
                w2_list = [(w2ps, 0), (w2ps, C)]
            else:
                w2psA = psump.tile([128, 512], f32, tag="w2", bufs=2, name=f"w2psA_{i}")
                w2psB = psump.tile([128, 512], f32, tag="w2", bufs=2, name=f"w2psB_{i}")
                for b, wp in ((0, w2psA), (1, w2psB)):
                    h, off = h_list[b]
                    nc.tensor.matmul(
                        wp[0:128, 0:C],
                        w2d_sb[i][:].bitcast(f32r),
                        h[:, off : off + C].bitcast(f32r),
                        start=True,
                        stop=True,
                    )
                w2_list = [(w2psA, 0), (w2psB, 0)]

            # ---- relu + reps + fold into next-layer prev features ----
            f_next = None
            if i < 4:
                M_out = C // 2
                f_next = fpool.tile([128, 2 * M_out], bf16, tag="f", name=f"f_{i}")
            for b in range(2):
                w2t, off = w2_list[b]
                if i < 4:
                    h2d = h2pool.tile([128, 512], f32, tag="h2", name=f"h2_{i}_{b}")
                    nc.scalar.activation(
                        h2d[:, 0:C],
                        w2t[0:128, off : off + C],
                        AF.Relu,
                        bias=b2d_sb[i][:],
                    )
                    ev = h2d[0:64, 0:C].rearrange("p (m two) -> p m two", two=2)
                    od = h2d[64:128, 0:C].rearrange("p (m two) -> p m two", two=2)
                    # interleaved layout: f[:, 2*m + b] = prev chunk m, sample b
                    fe = f_next[0:64, 0 : 2 * M_out].rearrange("p (m b) -> p m b", b=2)
                    fo = f_next[64:128, 0 : 2 * M_out].rearrange(
                        "p (m b) -> p m b", b=2
                    )
                    nc.vector.tensor_copy(fe[:, :, b : b + 1], ev[:, :, 0:1])
                    nc.vector.tensor_copy(fo[:, :, b : b + 1], od[:, :, 1:2])
                else:
                    h2d = h2pool.tile([128, 512], f32, tag="h2", name=f"h2_{i}_{b}")
                    nc.scalar.activation(
                        h2d[0:64, 0:C],
                        w2t[0:64, off : off + C],
                        AF.Relu,
                        bias=b2d_sb[i][0:64, :],
                    )
                nc.vector.tensor_reduce(
                    reps[:, 2 * i + b : 2 * i + b + 1],
                    h2d[0:64, 0:C],
                    axis=AX.X,
                    op=ALU.add,
                )
            if i < 4:
                f_prev = f_next

        # ---- tail: mult, feats, logit ----
        Lv = L_all[0:1, 0:10].rearrange("p (i b) -> p b i", b=2)
        mred = smallp.tile([1, 2], f32, tag="mred", name="mred")
        nc.vector.tensor_reduce(mred[:], Lv, axis=AX.X, op=ALU.add)
        mult = smallp.tile([1, 2], f32, tag="mult", name="mult")
        nc.scalar.activation(mult[:], mred[:], AF.Identity, bias=1.0, scale=0.5)

        featsA = constp.tile([128, 2], f32, tag="fA", name="featsA")
        featsB = constp.tile([128, 2], f32, tag="fB", name="featsB")
        featsC = constp.tile([65, 2], f32, tag="fC", name="featsC")
        nc.sync.dma_start(featsA[0:64, :], reps[:, 0:2])
        nc.sync.dma_start(featsA[64:128, :], reps[:, 2:4])
        nc.sync.dma_start(featsB[0:64, :], reps[:, 4:6])
        nc.sync.dma_start(featsB[64:128, :], reps[:, 6:8])
        nc.sync.dma_start(featsC[0:64, :], reps[:, 8:10])
        nc.sync.dma_start(featsC[64:65, :], mult[:])

        wrA = constp.tile([128, 1], f32, tag="wrA", name="wrA")
        wrB = constp.tile([128, 1], f32, tag="wrB", name="wrB")
        wrC = constp.tile([65, 1], f32, tag="wrC", name="wrC")
        nc.gpsimd.dma_start(wrA[:], wr_dram[0:128, :])
        nc.gpsimd.dma_start(wrB[:], wr_dram[128:256, :])
        nc.gpsimd.dma_start(wrC[:], wr_dram[256:321, :])
        brt = constp.tile([1, 1], f32, tag="br", name="brt")
        nc.gpsimd.dma_start(brt[:], br_dram[:])

        logps = psump.tile([1, 512], f32, tag="small", name="logps")
        nc.tensor.matmul(logps[0:1, 0:2], wrA[:], featsA[:], start=True, stop=False)
        nc.tensor.matmul(logps[0:1, 0:2], wrB[:], featsB[:], start=False, stop=False)
        nc.tensor.matmul(logps[0:1, 0:2], wrC[:], featsC[:], start=False, stop=True)
        outsb = smallp.tile([1, 2], f32, tag="outsb", name="outsb")
        nc.scalar.activation(
            outsb[:], logps[0:1, 0:2], AF.Identity, bias=brt[:], scale=1.0
        )
        nc.sync.dma_start(out_dram[:, 0:1], outsb[0:1, :])

    nc.compile()
    return nc


def _get_nc():
    if "nc" not in _CACHE:
        _CACHE["nc"] = _build()
    return _CACHE["nc"]


def make_in_maps(inputs):
    import ml_dtypes

    BF16 = ml_dtypes.bfloat16
    w1_bf = [np.ascontiguousarray(inputs[f"W1_{i}"].astype(BF16)) for i in range(5)]
    in_maps = []
    for core in range(N_CORES):
        s = core * B_LOC
        m = {}
        for i in range(5):
            m[f"p{i}"] = np.ascontiguousarray(
                inputs[f"p{i}"][s : s + B_LOC].astype(BF16)
            )
            m[f"W1_{i}"] = w1_bf[i]
            m[f"b1_{i}"] = inputs[f"b1_{i}"]
            m[f"W2_{i}"] = inputs[f"W2_{i}"]
            m[f"b2_{i}"] = inputs[f"b2_{i}"]
        m["Wr"] = inputs["Wr"]
        m["br"] = inputs["br"]
        in_maps.append(m)
    return in_maps


def run(inputs, trace=False):
    from concourse.bass_utils import run_bass_kernel_spmd

    nc = _get_nc()
    in_maps = make_in_maps(inputs)
    res = run_bass_kernel_spmd(nc, in_maps, list(range(N_CORES)), trace=trace)
    out = np.concatenate([r["out"] for r in res.results], axis=0)
    return out.astype(np.float32), res


def kernel(**inputs):
    inputs = {k: np.asarray(v, dtype=np.float32) for k, v in inputs.items()}
    out, _ = run(inputs, trace=False)
    return out
